# revision 25
# baseline (speedup 1.0000x reference)
"""BasicMFNet (embedding_lookup) Trainium2 kernel.

Reference computation:
    user_mask/item_mask = rows of embed_user/embed_item hit by indices
    pred  = (embed_user*user_mask) @ (embed_item*item_mask).T   [U, I] f32
    label = scatter-add of ratings at (indices[0], indices[1])  [U, I] f32
    ratio = U*I/NNZ  (python float)

Strategy (8 cores, tensor-parallel over items, per the sharding hint):
  * Only item columns [0, active_hi) can be nonzero (active_hi = max item
    index + 1 from the actual indices): item_mask is False and label has
    no entries beyond it.  The active range is split evenly across the 8
    cores (W columns each); columns >= 8*W are zero-filled on the host.
  * Host prep: masks applied to the embeddings, embeddings transposed to
    [H, *] (matmul lhsT/rhs layout), duplicate (u, i) pairs summed, and
    entries bucketed per [128 user x 512 item] tile into chunks of <=128.
  * Device per core (all memory-roofline work on chip):
      - pred shard [U, W] = eu_t.T @ ei_t on TensorE, PSUM -> SBUF (ACT)
        -> HBM, 512KB DMAs.
      - label shard [U, W] built densely via one-hot matmuls: for each
        chunk of <=128 entries, lhsT[e,u] = (iota_u == u_e) and
        rhs[e,i] = (iota_i == i_e) * val_e built on DVE (exact f32
        integer compares), accumulated into a [128,512] PSUM tile on
        TensorE, then ACT-copied and streamed out like pred.
        (An indirect-DMA / dma_scatter_add scatter was measured at
        60+ ns/token of Q7 descgen -- 1.5 ms/core; the matmul build rides
        the idle TensorE/DVE/GPSIMD under the ~190us DMA stream instead.)
"""

from contextlib import ExitStack

import numpy as np

import concourse.bacc as bacc
import concourse.mybir as mybir
import concourse.tile as tile
from concourse.bass_utils import run_bass_kernel_spmd

N_CORES = 8
H = 64          # hidden dim
U = 8192        # num users
I_FULL = 16384  # num items
P = 128         # partitions
IB = 512        # item sub-block (matmul moving free dim)


def _build_bass(W: int, cnts):
    """Bass program for one core.

    W: item-shard width (multiple of 512).  cnts[ub][ib]: number of
    <=128-entry chunks for label tile (ub, ib), uniform across cores."""
    f32 = mybir.dt.float32
    eq = mybir.AluOpType.is_equal
    mult = mybir.AluOpType.mult

    n_ublk = U // P
    n_iblk = W // IB
    nch = sum(sum(row) for row in cnts)

    nc = bacc.Bacc()
    emb_t = nc.dram_tensor("emb_t", [H, U + W], f32, kind="ExternalInput")
    # per-chunk entry columns: (u_local, i_local, val) f32 triples
    ck = nc.dram_tensor("ck", [P, nch * 3], f32, kind="ExternalInput")
    pred = nc.dram_tensor("pred", [U, W], f32, kind="ExternalOutput")
    label = nc.dram_tensor("label", [U, W], f32, kind="ExternalOutput")

    with ExitStack() as ctx:
        tc = ctx.enter_context(tile.TileContext(nc))
        const = ctx.enter_context(tc.tile_pool(name="const", bufs=1))
        # PSUM budget: 8 banks total; pred tile is W/512 banks, label 1 bank
        # x2 bufs -> keep ppsum bufs * W/512 <= 6
        ppsum_bufs = max(1, 6 // (W // IB))
        ppsum = ctx.enter_context(
            tc.tile_pool(name="ppsum", bufs=ppsum_bufs, space="PSUM")
        )
        lpsum = ctx.enter_context(tc.tile_pool(name="lpsum", bufs=2, space="PSUM"))
        stage = ctx.enter_context(tc.tile_pool(name="stage", bufs=4))
        lstage = ctx.enter_context(tc.tile_pool(name="lstage", bufs=4))
        ohu = ctx.enter_context(tc.tile_pool(name="ohu", bufs=3))
        rhs = ctx.enter_context(tc.tile_pool(name="rhs", bufs=3))

        emb_tile = const.tile([H, U + W], f32)
        nc.gpsimd.dma_start(out=emb_tile[:], in_=emb_t[:])
        eu_tile = emb_tile[:, :U]
        ei_tile = emb_tile[:, U:]

        ck_tile = const.tile([P, nch * 3], f32)
        nc.gpsimd.dma_start(out=ck_tile[:], in_=ck[:])

        iota = const.tile([P, IB], f32)
        nc.gpsimd.iota(
            iota[:], pattern=[[1, IB]], base=0, channel_multiplier=0,
            allow_small_or_imprecise_dtypes=True,
        )

        g = 0  # running chunk index
        for ub in range(n_ublk):
            # ---- pred tile [128, W] ----
            pp = ppsum.tile([P, W], f32, space="PSUM")
            for ib in range(n_iblk):
                nc.tensor.matmul(
                    pp[:, ib * IB:(ib + 1) * IB],
                    eu_tile[:, ub * P:(ub + 1) * P],
                    ei_tile[:, ib * IB:(ib + 1) * IB],
                    start=True,
                    stop=True,
                )
            pst = stage.tile([P, W], f32)
            nc.scalar.copy(pst[:], pp[:])
            nc.sync.dma_start(out=pred[ub * P:(ub + 1) * P, :], in_=pst[:])

            # ---- label tile [128, W] via one-hot matmuls ----
            lst = lstage.tile([P, W], f32)
            for ib in range(n_iblk):
                lp = lpsum.tile([P, IB], f32, space="PSUM")
                n = cnts[ub][ib]
                for k in range(n):
                    c = 3 * (g + k)
                    u_col = ck_tile[:, c:c + 1]
                    i_col = ck_tile[:, c + 1:c + 2]
                    v_col = ck_tile[:, c + 2:c + 3]
                    oh = ohu.tile([P, P], f32)
                    nc.vector.tensor_scalar(
                        out=oh[:], in0=iota[:, :P], scalar1=u_col,
                        scalar2=None, op0=eq,
                    )
                    rh = rhs.tile([P, IB], f32)
                    nc.vector.tensor_scalar(
                        out=rh[:], in0=iota[:], scalar1=i_col,
                        scalar2=v_col, op0=eq, op1=mult,
                    )
                    nc.tensor.matmul(
                        lp[:], oh[:], rh[:], start=(k == 0), stop=(k == n - 1)
                    )
                g += n
                nc.scalar.copy(lst[:, ib * IB:(ib + 1) * IB], lp[:])
            nc.sync.dma_start(out=label[ub * P:(ub + 1) * P, :], in_=lst[:])

    nc.finalize()
    return nc


def kernel(embed_user, embed_item, indices, ratings):
    embed_user = np.asarray(embed_user, dtype=np.float32)
    embed_item = np.asarray(embed_item, dtype=np.float32)
    indices = np.asarray(indices)
    ratings = np.asarray(ratings, dtype=np.float32)

    u_all = embed_user.shape[0]
    i_all = embed_item.shape[0]
    nnz = ratings.shape[0]
    assert u_all == U and i_all == I_FULL

    u_idx = indices[0].astype(np.int64)
    i_idx = indices[1].astype(np.int64)

    # masks (index preprocessing, applied to the embeddings on upload)
    user_mask = np.zeros(u_all, dtype=bool)
    user_mask[u_idx] = True
    item_mask = np.zeros(i_all, dtype=bool)
    item_mask[i_idx] = True

    # active item range, split across cores in W-wide shards
    active_hi = int(i_idx.max()) + 1 if nnz else 1
    W = max(IB, -(-active_hi // (N_CORES * IB)) * IB)  # ceil to mult of 512
    assert N_CORES * W <= I_FULL

    eu_m = embed_user * user_mask[:, None]
    ei_m = embed_item * item_mask[:, None]
    eu_t = np.ascontiguousarray(eu_m.T)                  # [H, U]
    ei_t_full = np.ascontiguousarray(ei_m.T)             # [H, I]

    # sum duplicate (u, i) pairs
    keys = u_idx * np.int64(I_FULL) + i_idx
    ukeys, inv = np.unique(keys, return_inverse=True)
    sums = np.bincount(inv, weights=ratings.astype(np.float64)).astype(np.float32)
    uu = (ukeys // I_FULL).astype(np.int64)
    ui = (ukeys % I_FULL).astype(np.int64)

    # bucket entries per core into [128u x 512i] tiles -> chunks of <=128
    n_ublk = U // P
    n_iblk = W // IB
    per_core = []
    counts = np.zeros((N_CORES, n_ublk, n_iblk), np.int64)
    for c in range(N_CORES):
        lo, hi = c * W, (c + 1) * W
        sel = (ui >= lo) & (ui < hi)
        uc, ic, vc = uu[sel], ui[sel] - lo, sums[sel]
        tu = uc // P
        ti = ic // IB
        order = np.argsort(tu * n_iblk + ti, kind="stable")
        uc, ic, vc, tu, ti = uc[order], ic[order], vc[order], tu[order], ti[order]
        np.add.at(counts[c], (tu, ti), 1)
        per_core.append((uc % P, ic % IB, vc, tu, ti))

    # uniform chunk schedule: per-tile max over cores, >= 1
    cnt_max = np.maximum(1, -(-counts.max(axis=0) // P))  # [n_ublk, n_iblk]
    cnts = cnt_max.tolist()
    nch = int(cnt_max.sum())
    starts = np.zeros((n_ublk, n_iblk), np.int64)
    starts.flat[1:] = np.cumsum(cnt_max.flat)[:-1]

    ck_all = []
    for c in range(N_CORES):
        ul, il, vl, tu, ti = per_core[c]
        ckd = np.zeros((P, nch, 3), np.float32)
        # entries of tile (a,b) occupy consecutive positions (sorted above)
        tile_ids = tu * n_iblk + ti
        bounds = np.searchsorted(tile_ids, np.arange(n_ublk * n_iblk + 1))
        for a in range(n_ublk):
            for b in range(n_iblk):
                t0, t1 = bounds[a * n_iblk + b], bounds[a * n_iblk + b + 1]
                m = t1 - t0
                if m == 0:
                    continue
                s = starts[a, b]
                pos = np.arange(m)
                ckd[pos % P, s + pos // P, 0] = ul[t0:t1]
                ckd[pos % P, s + pos // P, 1] = il[t0:t1]
                ckd[pos % P, s + pos // P, 2] = vl[t0:t1]
        ck_all.append(np.ascontiguousarray(ckd.reshape(P, nch * 3)))

    nc = _build_bass(W, cnts)
    in_maps = [
        {
            "emb_t": np.ascontiguousarray(
                np.concatenate([eu_t, ei_t_full[:, c * W:(c + 1) * W]], axis=1)
            ),
            "ck": ck_all[c],
        }
        for c in range(N_CORES)
    ]
    res = run_bass_kernel_spmd(nc, in_maps, core_ids=list(range(N_CORES))).results

    pred_full = np.zeros((u_all, i_all), dtype=np.float32)
    label_full = np.zeros((u_all, i_all), dtype=np.float32)
    for c in range(N_CORES):
        pred_full[:, c * W:(c + 1) * W] = res[c]["pred"]
        label_full[:, c * W:(c + 1) * W] = res[c]["label"]

    ratio = float(u_all * i_all) / float(nnz)
    return pred_full, label_full, ratio
